# revision 12
# baseline (speedup 1.0000x reference)
"""Trainium2 Bass kernel for additive (Bahdanau) attention GNN message passing.

score[n, m] = v . tanh(a[n] + b[m]);  w = softmax(score, axis=n)
ctx[m] = w[:, m].T @ x1;  out = tanh(concat([att, ctx_s, ctx_e]) @ W_lin.T + b)

tanh is replaced by a separable harmonic expansion
  tanh(s) ~ sum_k alpha_k sin(k w0 s),  k in {1,2,3,4,6}
so the O(N*M*H) nonlinearity becomes PE matmuls contracting h.  The HW ACT
Sin table is only valid for |arg| <~ 3.2, so the a-side basis is built from
three in-range ACT sins  s1 = sin(w0 a), c1 = sin(w0 a + pi/2),
s2 = sin(2 w0 a)  plus Squares and a short bf16 product chain; higher
harmonics are expanded as polynomials in these tiles and every
softmax-invariant constant term is dropped, giving 10 rhs "slots":
  s1, c1, s2, qs1=s1^2, qq=qs1^2, P23=s2*qs1, s3p=s1-4/3*P13, c3=c1-4*P31,
  s6p=c3*s3p, qc3=c3^2     (P13=s1*qs1, P31=c1*qs1)
with per-slot b-side lhsT combos  sum_j v*beta_j*(b harmonic tile)  folded
on the small side (beta absorbs alpha_k and expansion coefficients; the
v*beta columns come from a rank-1 PE matmul of single-row DMA constants).
Scores accumulate m-stationary into PSUM [m, n] via 30 512-wide matmuls
(start=True only on each bank's first matmul - start clears the whole
bank's has_written bits).  E = exp(score) is PE-transposed back to [n, m]
for the ctx matmuls; softmax sums fall out of a ones column in the x image.
Inputs arrive as packed bf16 images spread over 3 DMA queues (per-queue
DMA bandwidth is ~77GB/s, so parallelism and few triggers matter).
"""

import numpy as np
from ml_dtypes import bfloat16

import concourse.bass as bass
import concourse.tile as tile
from concourse import bacc, masks, mybir
from concourse.bass_utils import run_bass_kernel_spmd

F32 = mybir.dt.float32
F16 = mybir.dt.float16
BF16 = mybir.dt.bfloat16
AF = mybir.ActivationFunctionType
OP = mybir.AluOpType

H = 128
A = 256
N_S = 1024
N_E = 512
M = 1024
NC = 8
ML = M // NC
NT = N_S + N_E
NCH = NT // 128
CW = 129
X16W = NCH * CW          # 1548
IMG2W = X16W + 3 * A     # x16 | wlinT16

W0 = 0.267059
AL = {1: 1.17663, 2: 0.12087, 3: 0.17747, 4: 0.13768, 6: 0.13409}

# slot -> terms (k, trig, coef): lhsT = sum_j v * coef_j*alpha_kj * btile
# trig 0 => pairs the b-side cos tile, 1 => the b-side sin tile
SLOTS = (
    ("s1",  ((1, 0, 1.0),)),
    ("c1",  ((1, 1, 1.0),)),
    ("s2",  ((2, 0, 1.0), (4, 0, 2.0))),
    ("qs1", ((2, 1, -2.0), (4, 1, -8.0))),
    ("qq",  ((4, 1, 8.0),)),
    ("P23", ((4, 0, -4.0),)),
    ("s3p", ((3, 0, 3.0),)),
    ("c3",  ((3, 1, 1.0),)),
    ("s6p", ((6, 0, 6.0),)),
    ("qc3", ((6, 1, 2.0),)),
)
BETAS = [coef * AL[k] for _, terms in SLOTS for k, _, coef in terms]  # 12

PARTS = ((0, 512), (512, 1536))
BLOCKS = ((0, 512, 0), (512, 1024, 0), (1024, 1536, 1))  # (lo, hi, set)

_CACHE = {}


def _build():
    nc = bacc.Bacc(
        "TRN2", target_bir_lowering=False, debug=False, num_devices=NC
    )
    dr = lambda nm, sh: nc.dram_tensor(nm, sh, BF16, kind="ExternalInput").ap()
    d_imgC = dr("imgC", [128, 384])     # W2s.T | W2e.T | attT16
    d_imgD = dr("imgD", [128, 256])     # W1s.T | W1e.T
    d_b1 = dr("b1", [128, 512])         # stmtsT[:, 0:512]
    d_b2 = dr("b2", [128, 512])         # stmtsT[:, 512:1024]
    d_b3 = dr("b3", [128, 512])         # eresT
    d_crow = dr("crow", [1, 1536])      # vs|ve|bcs|bce|betas|blin
    d_img2 = dr("img2", [128, IMG2W])   # x16 | wlinT16
    d_out = nc.dram_tensor("out", [ML, A], F16, kind="ExternalOutput").ap()

    with tile.TileContext(nc) as tc:
        _emit(nc, tc, d_imgC, d_imgD, d_b1, d_b2, d_b3, d_crow, d_img2, d_out)

    nc.compile()
    return nc


def _emit(nc, tc, d_imgC, d_imgD, d_b1, d_b2, d_b3, d_crow, d_img2, d_out):
    from contextlib import ExitStack

    ctx = ExitStack()
    with ctx:
        const = ctx.enter_context(tc.tile_pool(name="const", bufs=1))
        bpool = ctx.enter_context(tc.tile_pool(name="bpool", bufs=1))
        apool = ctx.enter_context(tc.tile_pool(name="apool", bufs=1))
        ps_a = ctx.enter_context(
            tc.tile_pool(name="ps_a", bufs=1, space=bass.MemorySpace.PSUM))
        ps_score = ctx.enter_context(
            tc.tile_pool(name="ps_score", bufs=1, space=bass.MemorySpace.PSUM))
        ps_small = ctx.enter_context(
            tc.tile_pool(name="ps_small", bufs=1, space=bass.MemorySpace.PSUM))

        # ---- init + table warm ----
        ident16 = const.tile([128, 128], BF16)
        masks.make_identity(nc, ident16[:])
        ones16 = const.tile([1, 128], BF16)
        nc.gpsimd.memset(ones16[:], 1.0)
        pz = const.tile([128, 1], F32)
        nc.gpsimd.memset(pz[:], 1.5707963267948966)
        scratch = const.tile([128, 1], F32)
        nc.gpsimd.memset(scratch[:], 0.0)

        # ---- DMAs spread over 3 queues (scalar-queue triggers first so the
        # ACT table hoisting isn't split around them) ----
        sb_imgD = const.tile([128, 256], BF16)
        nc.scalar.dma_start(sb_imgD[:], d_imgD[:, :])
        sb_b1 = const.tile([128, 512], BF16)
        nc.scalar.dma_start(sb_b1[:], d_b1[:, :])
        nc.scalar.activation(scratch[:], scratch[:], AF.Sin)
        sb_imgC = const.tile([128, 384], BF16)
        nc.sync.dma_start(sb_imgC[:], d_imgC[:, :])
        sb_crow = const.tile([1, 1536], BF16)
        nc.sync.dma_start(sb_crow[0:1, :], d_crow[0:1, :])
        sb_b2 = const.tile([128, 512], BF16)
        nc.gpsimd.dma_start(sb_b2[:], d_b2[:, :])
        sb_b3 = const.tile([128, 512], BF16)
        nc.gpsimd.dma_start(sb_b3[:], d_b3[:, :])
        sb_img2 = const.tile([128, IMG2W], BF16)
        nc.gpsimd.dma_start(sb_img2[:], d_img2[:, :])

        attT16 = sb_imgC[:, 256:384]
        crow = lambda r, n: sb_crow[0:1, r * 256:r * 256 + n]
        x16 = sb_img2[:, 0:X16W]
        wlin = lambda j: sb_img2[:, X16W + j * A:X16W + (j + 1) * A]

        # ---- front PE: bT + bias + v*beta rank-1, one PSUM bank ----
        ps_bT = ps_small.tile([128, 288], F32, tag="ctx", name="ps_bT")
        nc.tensor.matmul(ps_bT[:, 0:ML], sb_imgC[:, 0:128], attT16,
                         start=True, stop=False, skip_group_check=True)
        nc.tensor.matmul(ps_bT[:, ML:2 * ML], sb_imgC[:, 128:256], attT16,
                         start=False, stop=False, skip_group_check=True)
        nc.tensor.matmul(ps_bT[:, 0:ML], crow(2, 128), ones16[0:1, :],
                         start=False, stop=False, skip_group_check=True)
        nc.tensor.matmul(ps_bT[:, ML:2 * ML], crow(3, 128), ones16[0:1, :],
                         start=False, stop=False, skip_group_check=True)
        NB = len(BETAS)
        nc.tensor.matmul(ps_bT[:, 256:256 + NB], crow(0, 128), crow(4, NB),
                         start=False, stop=False, skip_group_check=True)
        nc.tensor.matmul(ps_bT[:, 256 + NB:256 + 2 * NB], crow(1, 128),
                         crow(4, NB), start=False, stop=True,
                         skip_group_check=True)
        sb_vbeta = const.tile([128, 2 * NB], F32)
        nc.vector.tensor_copy(sb_vbeta[:], ps_bT[:, 256:256 + 2 * NB])

        # ---- aT -> [128, 1536] PSUM ----
        ps_aT = ps_a.tile([128, NT], F32, tag="aT", name="ps_aT")
        nc.tensor.matmul(ps_aT[:, 0:512], sb_imgD[:, 0:128], sb_b1[:],
                         start=True, stop=True)
        nc.tensor.matmul(ps_aT[:, 512:1024], sb_imgD[:, 0:128], sb_b2[:],
                         start=True, stop=True)
        nc.tensor.matmul(ps_aT[:, 1024:1536], sb_imgD[:, 128:256], sb_b3[:],
                         start=True, stop=True)

        # att + b_lin parts of the final linear
        ps_out = ps_small.tile([128, 400], F32, tag="out")
        nc.tensor.matmul(ps_out[:, 0:A], attT16, wlin(0),
                         start=True, stop=False, skip_group_check=True)
        nc.tensor.matmul(ps_out[:, 0:A], ones16[0:1, :], crow(5, A),
                         start=False, stop=False, skip_group_check=True)

        # ---- b-side basis (small): ACT sins + Pool/DVE chain ----
        # u1b reads cols 0:280 (incl. junk v*beta cols) so ACT's first
        # PSUM-bank read orders after all PE writes (collision avoidance)
        u1b = bpool.tile([128, 288], BF16, name="u1b")
        nc.scalar.activation(u1b[:, 0:280], ps_bT[:, 0:280], AF.Sin,
                             scale=0.5 * W0)
        bt = {k: bpool.tile([128, 512], BF16, name=f"bt{k}") for k in AL}
        nc.scalar.activation(bt[1][:, 0:256], ps_bT[:, 0:256], AF.Sin,
                             scale=W0)
        s1b = bt[1][:, 0:256]

        def btmp(nm):
            return bpool.tile([128, 256], BF16, name=nm)[:]

        g = nc.gpsimd
        q1b = btmp("q1b")
        g.tensor_tensor(q1b, u1b[:, 0:256], u1b[:, 0:256], OP.mult)
        c1b = bt[1][:, 256:512]
        nc.vector.tensor_scalar(c1b, q1b, -2.0, 1.0, OP.mult, OP.add)
        c1twob = btmp("c1twob")
        nc.vector.tensor_scalar_mul(c1twob, c1b, 2.0)
        g.tensor_tensor(bt[2][:, 0:256], c1twob, s1b, OP.mult)      # s2b
        qs1b = btmp("qs1b")
        g.tensor_tensor(qs1b, s1b, s1b, OP.mult)
        c2b = bt[2][:, 256:512]
        nc.vector.tensor_scalar(c2b, qs1b, -2.0, 1.0, OP.mult, OP.add)
        c2twob = btmp("c2twob")
        nc.vector.tensor_scalar_mul(c2twob, c2b, 2.0)
        c2mb = btmp("c2mb")
        nc.vector.tensor_scalar_sub(c2mb, c2twob, 1.0)
        g.tensor_tensor(bt[3][:, 256:512], c1b, c2mb, OP.mult)      # c3b
        s3tb = btmp("s3tb")
        g.tensor_tensor(s3tb, c1twob, bt[2][:, 0:256], OP.mult)
        g.tensor_tensor(bt[3][:, 0:256], s3tb, s1b, OP.subtract)    # s3b
        g.tensor_tensor(bt[4][:, 0:256], c2twob, bt[2][:, 0:256],
                        OP.mult)                                    # s4b
        qs2b = btmp("qs2b")
        g.tensor_tensor(qs2b, bt[2][:, 0:256], bt[2][:, 0:256], OP.mult)
        nc.vector.tensor_scalar(bt[4][:, 256:512], qs2b, -2.0, 1.0,
                                OP.mult, OP.add)                    # c4b
        c3twob = btmp("c3twob")
        nc.vector.tensor_scalar_mul(c3twob, bt[3][:, 256:512], 2.0)
        g.tensor_tensor(bt[6][:, 0:256], c3twob, bt[3][:, 0:256],
                        OP.mult)                                    # s6b
        qc3b = btmp("qc3b")
        g.tensor_tensor(qc3b, bt[3][:, 256:512], bt[3][:, 256:512], OP.mult)
        nc.vector.tensor_scalar(bt[6][:, 256:512], qc3b, 2.0, -1.0,
                                OP.mult, OP.add)                    # c6b

        # ---- slot lhsT combos: w = sum_j (v*beta_j) * btile_j ----
        # single terms on Pool ((AP, imm, mult, mult) form), second term of
        # the 2-term combos via DVE scalar_tensor_tensor with an AP scalar
        wsl = {}
        bi = 0
        for nm, terms in SLOTS:
            wsl[nm] = bpool.tile([128, 256], BF16, name=f"w_{nm}")
            for st in range(2):
                k0, tr0, _ = terms[0]
                src0 = bt[k0][:, (1 - tr0) * 256 + st * 128:
                              (1 - tr0) * 256 + st * 128 + 128]
                dst = wsl[nm][:, st * 128:st * 128 + 128]
                if len(terms) == 1:
                    nc.vector.tensor_scalar(
                        dst, src0,
                        sb_vbeta[:, st * NB + bi:st * NB + bi + 1],
                        None, OP.mult)
                else:
                    t0 = bpool.tile([128, 128], BF16, name=f"wt_{nm}{st}")
                    nc.vector.tensor_scalar(
                        t0[:], src0,
                        sb_vbeta[:, st * NB + bi:st * NB + bi + 1],
                        None, OP.mult)
                    k1_, tr1, _ = terms[1]
                    src1 = bt[k1_][:, (1 - tr1) * 256 + st * 128:
                                   (1 - tr1) * 256 + st * 128 + 128]
                    nc.vector.scalar_tensor_tensor(
                        dst, src1,
                        sb_vbeta[:, st * NB + bi + 1:st * NB + bi + 2],
                        t0[:], OP.mult, OP.add)
            bi += len(terms)

        # ---- a-side basis ----
        at = {}
        for nm in ("s1", "c1", "s2", "qs1", "qq", "P13", "P31", "P23",
                   "s3p", "c3", "s6p", "qc3"):
            at[nm] = apool.tile([128, NT], BF16, name=nm)

        def act1(out, in_, func, p, scale=1.0, bias=0.0):
            lo, hi = PARTS[p]
            nc.scalar.activation(out[:, lo:hi], in_[:, lo:hi], func,
                                 scale=scale, bias=bias)

        def dve_tt1(out, in0, in1, op, p):
            lo, hi = PARTS[p]
            nc.vector.tensor_tensor(out[:, lo:hi], in0[:, lo:hi],
                                    in1[:, lo:hi], op)

        def dve_stt1(out, in0, sc, in1, op0, op1, p):
            lo, hi = PARTS[p]
            nc.vector.scalar_tensor_tensor(out[:, lo:hi], in0[:, lo:hi],
                                           sc, in1[:, lo:hi], op0, op1)

        # ACT: part-0 sins first so the part-0 DVE chain starts early
        for p in range(2):
            act1(at["s1"][:], ps_aT[:], AF.Sin, p, W0)
            act1(at["c1"][:], ps_aT[:], AF.Sin, p, W0, pz[:, 0:1])
            act1(at["qs1"][:], at["s1"][:], AF.Square, p)
            act1(at["s2"][:], ps_aT[:], AF.Sin, p, 2 * W0)
        for p in range(2):
            act1(at["qq"][:], at["qs1"][:], AF.Square, p)
        # DVE products: full part-0 chain, then part-1
        for p in range(2):
            dve_tt1(at["P13"][:], at["s1"][:], at["qs1"][:], OP.mult, p)
            dve_tt1(at["P31"][:], at["c1"][:], at["qs1"][:], OP.mult, p)
            dve_tt1(at["P23"][:], at["s2"][:], at["qs1"][:], OP.mult, p)
            dve_stt1(at["c3"][:], at["P31"][:], -4.0, at["c1"][:],
                     OP.mult, OP.add, p)
            dve_stt1(at["s3p"][:], at["P13"][:], -4.0 / 3.0, at["s1"][:],
                     OP.mult, OP.add, p)
            act1(at["qc3"][:], at["c3"][:], AF.Square, p)
            dve_tt1(at["s6p"][:], at["c3"][:], at["s3p"][:], OP.mult, p)

        # ---- scores, m-stationary: ps_sc[m, n] ----
        ps_sc = ps_score.tile([128, NT], F32)
        for lo, hi, st in BLOCKS:
            for si, (nm, _) in enumerate(SLOTS):
                nc.tensor.matmul(
                    ps_sc[:, lo:hi], wsl[nm][:, st * 128:st * 128 + 128],
                    at[nm][:, lo:hi],
                    start=(si == 0), stop=(si == len(SLOTS) - 1),
                    skip_group_check=True)

        # ---- epilogue ----
        E_mT = apool.tile([128, NT], BF16, name="E_mT")
        for lo, hi, _ in BLOCKS:
            nc.scalar.activation(E_mT[:, lo:hi], ps_sc[:, lo:hi], AF.Exp)

        ps_tr2 = ps_a.tile([128, 2 * NT], BF16, tag="aT", name="ps_tr2")
        sb_E = apool.tile([128, NT], BF16, name="sb_E")
        for c in range(NCH):
            nc.tensor.matmul(ps_tr2[:, c * 128:(c + 1) * 128],
                             E_mT[:, c * 128:(c + 1) * 128], ident16[:],
                             is_transpose=True)
            if c == 7:
                nc.vector.tensor_copy(sb_E[:, 0:1024], ps_tr2[:, 0:1024])
        nc.vector.tensor_copy(sb_E[:, 1024:1536], ps_tr2[:, 1024:1536])

        ps_ctx = ps_small.tile([128, 288], F32, tag="ctx", name="ps_ctx")
        for c in range(8):
            nc.tensor.matmul(ps_ctx[:, 0:CW],
                             sb_E[:, c * 128:(c + 1) * 128],
                             x16[:, c * CW:(c + 1) * CW],
                             start=(c == 0), stop=(c == 7))
        for c in range(8, 12):
            nc.tensor.matmul(ps_out[:, 264:264 + CW],
                             sb_E[:, c * 128:(c + 1) * 128],
                             x16[:, c * CW:(c + 1) * CW],
                             start=False, stop=False, skip_group_check=True)

        sb_recip = apool.tile([128, 2], F32, name="recip")
        nc.vector.reciprocal(sb_recip[:, 0:1], ps_ctx[:, H:H + 1])
        nc.vector.reciprocal(sb_recip[:, 1:2], ps_out[:, 392:393])
        sb_ctx = apool.tile([128, 2 * H], BF16, name="sb_ctx")
        nc.vector.tensor_scalar(sb_ctx[:, 0:H], ps_ctx[:, 0:H],
                                sb_recip[:, 0:1], None, OP.mult)
        nc.vector.tensor_scalar(sb_ctx[:, H:2 * H], ps_out[:, 264:392],
                                sb_recip[:, 1:2], None, OP.mult)

        ps_tr3 = ps_a.tile([128, 2 * NT], BF16, tag="aT", name="ps_tr3")
        nc.tensor.matmul(ps_tr3[:, 0:128], sb_ctx[:, 0:H], ident16[:],
                         is_transpose=True)
        nc.tensor.matmul(ps_tr3[:, 128:256], sb_ctx[:, H:2 * H], ident16[:],
                         is_transpose=True)
        sb_ctxT = apool.tile([128, 2 * H], BF16, name="sb_ctxT")
        nc.vector.tensor_copy(sb_ctxT[:], ps_tr3[:, 0:256])

        nc.tensor.matmul(ps_out[:, 0:A], sb_ctxT[:, 0:H], wlin(1),
                         start=False, stop=False, skip_group_check=True)
        nc.tensor.matmul(ps_out[:, 0:A], sb_ctxT[:, H:2 * H], wlin(2),
                         start=False, stop=True, skip_group_check=True)

        sb_out = apool.tile([128, A], F16, name="sb_out")
        nc.scalar.activation(sb_out[:], ps_out[:, 0:A], AF.Tanh)
        nc.sync.dma_start(d_out[0:48, :], sb_out[0:48, :])
        nc.scalar.dma_start(d_out[48:96, :], sb_out[48:96, :])
        nc.gpsimd.dma_start(d_out[96:128, :], sb_out[96:128, :])


def _get_nc():
    if "nc" not in _CACHE:
        _CACHE["nc"] = _build()
    return _CACHE["nc"]


def _prep_inputs(inputs):
    """Host-side layout prep: transposes / bf16 casts / image packing."""
    f = {k: np.ascontiguousarray(np.asarray(v, np.float32))
         for k, v in inputs.items()}
    stmts, eres = f["attendee_stmts"], f["attendee_eres"]
    ws, we, wlin = f["Ws_concat"], f["We_concat"], f["W_lin"]

    stT = stmts.T
    imgD = np.concatenate([ws[:, :H].T, we[:, :H].T], axis=1)

    crow = np.zeros((1, 1536), np.float32)
    crow[0, 0:128] = f["vs_single"]
    crow[0, 256:384] = f["ve_single"]
    crow[0, 512:640] = f["bs_concat"]
    crow[0, 768:896] = f["be_concat"]
    crow[0, 1024:1024 + len(BETAS)] = np.asarray(BETAS, np.float32)
    crow[0, 1280:1536] = f["b_lin"]

    x = np.empty((128, X16W), np.float32)
    for c in range(8):
        x[:, c * CW:c * CW + H] = stmts[c * 128:(c + 1) * 128]
        x[:, c * CW + H] = 1.0
    for c in range(8, 12):
        x[:, c * CW:c * CW + H] = eres[(c - 8) * 128:(c - 7) * 128]
        x[:, c * CW + H] = 1.0
    wlinT = np.concatenate(
        [wlin[:, 0:H].T, wlin[:, H:2 * H].T, wlin[:, 2 * H:3 * H].T], axis=1)
    img2 = np.concatenate([x, wlinT], axis=1)

    cb = lambda a_: np.ascontiguousarray(a_.astype(bfloat16))
    shared = {
        "imgD": cb(imgD), "b1": cb(stT[:, 0:512]), "b2": cb(stT[:, 512:1024]),
        "b3": cb(eres.T), "crow": cb(crow), "img2": cb(img2),
    }
    att = f["attender"]
    in_maps = []
    for i in range(NC):
        imgC = np.concatenate(
            [ws[:, H:].T, we[:, H:].T, att[i * ML:(i + 1) * ML].T], axis=1)
        in_maps.append(dict(shared, imgC=cb(imgC)))
    return in_maps


def kernel(**inputs) -> np.ndarray:
    nc = _get_nc()
    in_maps = _prep_inputs(inputs)
    res = run_bass_kernel_spmd(nc, in_maps, list(range(NC)))
    return np.concatenate(
        [res.results[i]["out"].astype(np.float32) for i in range(NC)], axis=0)
